# revision 7
# baseline (speedup 1.0000x reference)
"""Trainium2 Bass kernel for nn_CombineRadialSpeciesWithAngular.

Per-angular-order GEMM out_l = v_l @ W[l], flattened+concatenated over l.
Full shapes: v_l [20000, 2l+1, 128] f32 (l=0..5), W [6, 128, 256] f32,
out [720000, 256] f32.

Strategy (8 NeuronCores, data-parallel over samples):
  - Each core gets 2500 samples of every block -> 90000 output rows.
  - Host pre-transposes each core's rows into vt [128, 90000] bf16
    (contraction dim p on partitions, rows in natural block order so
    supertile s of 2500 rows is l-homogeneous with l = isqrt(s)).
  - Device: stationary = W[l] half [128,128] bf16 (FWL weight loads),
    moving = 500-row slices of vt -> PSUM [128,500] f32 -> DVE/ACT copy
    to bf16 -> out_t [2,128,90000] bf16 (combined dim on partitions).
  - Host un-transposes out_t -> [90000, 256] f32 and re-concatenates.

Why this layout: every DMA (in and out) is 128 descriptors of 10 KB.
The HWDGE splits a transfer across SDMA engines only in equal parts, so
a 125-descriptor transfer lands on 5 engines (26 GB/s each) while a
128-descriptor one uses all 16. The f32 predecessor of this kernel was
DMA-bound at 170 GB/s effective for exactly that reason. bf16 I/O
additionally halves the bytes: 69 MB/core vs 138 MB/core, ~3e-3 rel
err from input/output rounding (gate is 2e-2). Input DMAs ride the
ACT HWDGE ring, output DMAs the SP ring, so the streams overlap.

Uses bacc.Bacc (not bass.Bass): its compile pipeline legalizes semaphore
waits to this target's 1-wait-per-instruction limit; plain Bass output
fails walrus codegen ("Too many sync wait commands").
"""

import math
import sys

import numpy as np

for _p in ("/opt/trn_rl_repo", "/root/.axon_site/_ro/trn_rl_repo"):
    if _p not in sys.path:
        sys.path.append(_p)

import ml_dtypes

import concourse.bacc as bacc
import concourse.mybir as mybir
import concourse.tile as tile
from concourse.bass_utils import run_bass_kernel_spmd

N_CORES = 8
N_SAMPLES = 20000
N_PROPS = 128
N_COMB = 256
N_ANG = 6
S_CORE = N_SAMPLES // N_CORES          # 2500 samples per core
M_TOTAL = sum(2 * l + 1 for l in range(N_ANG))  # 36
ROWS = S_CORE * M_TOTAL                # 90000 rows per core
SUP = 2500                             # rows per supertile (l-homogeneous)
NSUP = ROWS // SUP                     # 36
# Row-blocks per supertile: PSUM-bank-native sizes (512 f32 = one 2 KiB
# bank) so matmul outputs and copies are contiguous and bank-aligned.
BLOCKS = [(0, 512), (512, 512), (1024, 512), (1536, 512), (2048, 452)]
GROUPS = ((0, 1), (2, 3), (4,))        # blocks per PSUM tile / copy
PAIR = 2 * SUP                         # rows per input DMA (1.28 MB bf16)

F32 = mybir.dt.float32
BF16 = mybir.dt.bfloat16
NP_BF16 = ml_dtypes.bfloat16

_nc_cache = {}


def build_nc(reps=1):
    """reps>1 repeats the whole body inside one NEFF (profiling only)."""
    if reps in _nc_cache:
        return _nc_cache[reps]

    nc = bacc.Bacc()
    vt = nc.dram_tensor("vt", [128, ROWS], BF16, kind="ExternalInput")
    # w2[p, 2l+h, c] = W[l][p, 128h+c]
    w2 = nc.dram_tensor("w2", [128, 2 * N_ANG, 128], BF16, kind="ExternalInput")
    # out_t[h, c, r] = out[r, 128h+c]
    out = nc.dram_tensor("out", [2, 128, ROWS], BF16, kind="ExternalOutput")

    with tile.TileContext(nc) as tc:
        with (
            tc.tile_pool(name="wp", bufs=1) as wp,
            tc.tile_pool(name="vp", bufs=3) as vp,
            tc.tile_pool(name="op", bufs=2) as op,
            tc.tile_pool(name="pp", bufs=3, space="PSUM") as pp,
        ):
            wt = wp.tile([128, 2 * N_ANG, 128], BF16)
            nc.sync.dma_start(wt[:], w2[:])

            for rep in range(reps):
                for sp in range(NSUP // 2):           # supertile pairs
                    vt_t = vp.tile([128, PAIR], BF16)
                    nc.scalar.dma_start(
                        vt_t[:], vt[:, sp * PAIR:(sp + 1) * PAIR])

                    for half in range(2):
                        s = 2 * sp + half
                        l = math.isqrt(s)             # block bounds at squares
                        base = half * SUP
                        for h in range(2):
                            ot = op.tile([128, SUP], BF16, name=f"ot{h}")
                            for gi, grp in enumerate(GROUPS):
                                ps = pp.tile([128, len(grp), 512], F32)
                                for q, b in enumerate(grp):
                                    off, n = BLOCKS[b]
                                    nc.tensor.matmul(
                                        ps[:, q, :n],
                                        wt[:, 2 * l + h, :],
                                        vt_t[:, base + off:base + off + n],
                                        start=True, stop=True)
                                off0 = BLOCKS[grp[0]][0]
                                tot = sum(BLOCKS[b][1] for b in grp)
                                dst = ot[:, off0:off0 + tot]
                                src = (ps[:, :, :] if tot == 1024
                                       else ps[:, 0, :tot])
                                # balance PSUM drain: DVE ~123 G elem/s,
                                # ACT ~114 G elem/s (cayman errata rates)
                                use_dve = (gi == 0) if gi < 2 else (
                                    (s + h) % 2 == 0)
                                if use_dve:
                                    nc.vector.tensor_copy(dst, src)
                                else:
                                    nc.scalar.copy(dst, src)
                            # per-half-supertile 640 KB out DMA; alternate
                            # HWDGE rings so each carries ~34.5 MB total
                            eng = nc.sync if (h == 0) == (sp % 2 == 0) \
                                else nc.scalar
                            eng.dma_start(
                                out[h, :, s * SUP:(s + 1) * SUP], ot[:])

    nc.finalize()  # Bacc compile: wait legalization + reg alloc
    _nc_cache[reps] = nc
    return nc


def shard_inputs(inputs):
    """Full f32 inputs -> per-core bf16 in_maps (host transpose + cast)."""
    w = np.asarray(inputs["W"], dtype=np.float32)          # [6,128,256]
    w2 = np.ascontiguousarray(
        w.transpose(1, 0, 2).reshape(128, N_ANG, 2, 128)
        .reshape(128, 2 * N_ANG, 128)).astype(NP_BF16)
    in_maps = []
    for i in range(N_CORES):
        vt_i = np.empty((128, ROWS), dtype=np.float32)
        col = 0
        for l in range(N_ANG):
            n = S_CORE * (2 * l + 1)
            blk = np.asarray(inputs[f"values_l{l}"][i * S_CORE:(i + 1) * S_CORE],
                             dtype=np.float32)
            vt_i[:, col:col + n] = blk.reshape(n, 128).T
            col += n
        in_maps.append({"vt": vt_i.astype(NP_BF16), "w2": w2})
    return in_maps


def unshard_output(core_outs):
    """Per-core [2, 128, 90000] bf16 -> full [720000, 256] f32."""
    full = np.empty((N_SAMPLES * M_TOTAL, N_COMB), dtype=np.float32)
    for i, o in enumerate(core_outs):
        # o[h, c, r] = out[r, 128h+c] -> [90000, 256]
        o = np.asarray(o).reshape(N_COMB, ROWS).T.astype(np.float32)
        for l in range(N_ANG):
            n = S_CORE * (2 * l + 1)
            src0 = S_CORE * l * l                      # local block offset
            dst0 = N_SAMPLES * l * l + i * n           # global block offset
            full[dst0:dst0 + n] = o[src0:src0 + n]
    return full


def run_sharded(in_maps, **kwargs):
    nc = build_nc()
    return run_bass_kernel_spmd(nc, in_maps, core_ids=list(range(N_CORES)),
                                **kwargs)


def kernel(**inputs):
    res = run_sharded(shard_inputs(inputs))
    return unshard_output([r["out"] for r in res.results])


# revision 8
# speedup vs baseline: 1.1130x; 1.1130x over previous
"""Trainium2 Bass kernel for nn_CombineRadialSpeciesWithAngular.

Per-angular-order GEMM out_l = v_l @ W[l], flattened+concatenated over l.
Full shapes: v_l [20000, 2l+1, 128] f32 (l=0..5), W [6, 128, 256] f32,
out [720000, 256] f32.

Strategy (8 NeuronCores, data-parallel over samples):
  - Each core gets 2500 samples of every block -> 90000 output rows.
  - Host pre-transposes each core's rows into vt [128, 90000] bf16
    (contraction dim p on partitions, rows in natural block order so
    supertile s of 2500 rows is l-homogeneous with l = isqrt(s)).
  - Device: stationary = W[l] half [128,128] bf16 (FWL weight loads),
    moving = 500-row slices of vt -> PSUM [128,500] f32 -> DVE/ACT copy
    to bf16 -> out_t [2,128,90000] bf16 (combined dim on partitions).
  - Host un-transposes out_t -> [90000, 256] f32 and re-concatenates.

Why this layout: every DMA (in and out) is 128 descriptors of 10 KB.
The HWDGE splits a transfer across SDMA engines only in equal parts, so
a 125-descriptor transfer lands on 5 engines (26 GB/s each) while a
128-descriptor one uses all 16. The f32 predecessor of this kernel was
DMA-bound at 170 GB/s effective for exactly that reason. bf16 I/O
additionally halves the bytes: 69 MB/core vs 138 MB/core, ~3e-3 rel
err from input/output rounding (gate is 2e-2). Input DMAs ride the
ACT HWDGE ring, output DMAs the SP ring, so the streams overlap.

Uses bacc.Bacc (not bass.Bass): its compile pipeline legalizes semaphore
waits to this target's 1-wait-per-instruction limit; plain Bass output
fails walrus codegen ("Too many sync wait commands").
"""

import math
import sys

import numpy as np

for _p in ("/opt/trn_rl_repo", "/root/.axon_site/_ro/trn_rl_repo"):
    if _p not in sys.path:
        sys.path.append(_p)

import ml_dtypes

import concourse.bacc as bacc
import concourse.mybir as mybir
import concourse.tile as tile
from concourse.bass_utils import run_bass_kernel_spmd

N_CORES = 8
N_SAMPLES = 20000
N_PROPS = 128
N_COMB = 256
N_ANG = 6
S_CORE = N_SAMPLES // N_CORES          # 2500 samples per core
M_TOTAL = sum(2 * l + 1 for l in range(N_ANG))  # 36
ROWS = S_CORE * M_TOTAL                # 90000 rows per core
SUP = 2500                             # rows per supertile (l-homogeneous)
NSUP = ROWS // SUP                     # 36
# Row-blocks per supertile: PSUM-bank-native sizes (512 f32 = one 2 KiB
# bank) so matmul outputs and copies are contiguous and bank-aligned.
BLOCKS = [(0, 512), (512, 512), (1024, 512), (1536, 512), (2048, 452)]
GROUPS = ((0, 1), (2, 3), (4,))        # blocks per PSUM tile / copy
PAIR = 2 * SUP                         # rows per input DMA (1.28 MB bf16)

F32 = mybir.dt.float32
BF16 = mybir.dt.bfloat16
NP_BF16 = ml_dtypes.bfloat16

_nc_cache = {}


def build_nc(reps=1):
    """reps>1 repeats the whole body inside one NEFF (profiling only)."""
    if reps in _nc_cache:
        return _nc_cache[reps]

    nc = bacc.Bacc()
    vt = nc.dram_tensor("vt", [128, ROWS], BF16, kind="ExternalInput")
    # w2[p, 2l+h, c] = W[l][p, 128h+c]
    w2 = nc.dram_tensor("w2", [128, 2 * N_ANG, 128], BF16, kind="ExternalInput")
    # out_t[h, c, r] = out[r, 128h+c]
    out = nc.dram_tensor("out", [2, 128, ROWS], BF16, kind="ExternalOutput")

    with tile.TileContext(nc) as tc:
        with (
            tc.tile_pool(name="wp", bufs=1) as wp,
            tc.tile_pool(name="vp", bufs=3) as vp,
            tc.tile_pool(name="op", bufs=2) as op,
            tc.tile_pool(name="pp", bufs=3, space="PSUM") as pp,
        ):
            wt = wp.tile([128, 2 * N_ANG, 128], BF16)
            nc.sync.dma_start(wt[:], w2[:])

            for rep in range(reps):
                for sp in range(NSUP // 2):           # supertile pairs
                    vt_t = vp.tile([128, PAIR], BF16)
                    nc.scalar.dma_start(
                        vt_t[:], vt[:, sp * PAIR:(sp + 1) * PAIR])

                    ot = [op.tile([128, PAIR], BF16, name=f"ot{h}")
                          for h in range(2)]
                    for half in range(2):
                        s = 2 * sp + half
                        l = math.isqrt(s)             # block bounds at squares
                        base = half * SUP
                        for h in range(2):
                            for gi, grp in enumerate(GROUPS):
                                ps = pp.tile([128, len(grp), 512], F32)
                                for q, b in enumerate(grp):
                                    off, n = BLOCKS[b]
                                    nc.tensor.matmul(
                                        ps[:, q, :n],
                                        wt[:, 2 * l + h, :],
                                        vt_t[:, base + off:base + off + n],
                                        start=True, stop=True)
                                off0 = BLOCKS[grp[0]][0]
                                tot = sum(BLOCKS[b][1] for b in grp)
                                dst = ot[h][:, base + off0:base + off0 + tot]
                                src = (ps[:, :, :] if tot == 1024
                                       else ps[:, 0, :tot])
                                # balance PSUM drain: DVE ~123 G elem/s,
                                # ACT ~114 G elem/s (cayman errata rates).
                                # All out DMAs stay on the idle SP engine:
                                # a dma_start on ACT would stall ACT's
                                # strict-FIFO queue at the wait-for-copy
                                # semaphore, blocking its own copies.
                                use_dve = (gi == 0) if gi < 2 else (
                                    (s + h) % 2 == 0)
                                if use_dve:
                                    nc.vector.tensor_copy(dst, src)
                                else:
                                    nc.scalar.copy(dst, src)

                    for h in range(2):
                        nc.sync.dma_start(
                            out[h, :, sp * PAIR:(sp + 1) * PAIR], ot[h][:])

    nc.finalize()  # Bacc compile: wait legalization + reg alloc
    _nc_cache[reps] = nc
    return nc


def shard_inputs(inputs):
    """Full f32 inputs -> per-core bf16 in_maps (host transpose + cast)."""
    w = np.asarray(inputs["W"], dtype=np.float32)          # [6,128,256]
    w2 = np.ascontiguousarray(
        w.transpose(1, 0, 2).reshape(128, N_ANG, 2, 128)
        .reshape(128, 2 * N_ANG, 128)).astype(NP_BF16)
    in_maps = []
    for i in range(N_CORES):
        vt_i = np.empty((128, ROWS), dtype=np.float32)
        col = 0
        for l in range(N_ANG):
            n = S_CORE * (2 * l + 1)
            blk = np.asarray(inputs[f"values_l{l}"][i * S_CORE:(i + 1) * S_CORE],
                             dtype=np.float32)
            vt_i[:, col:col + n] = blk.reshape(n, 128).T
            col += n
        in_maps.append({"vt": vt_i.astype(NP_BF16), "w2": w2})
    return in_maps


def unshard_output(core_outs):
    """Per-core [2, 128, 90000] bf16 -> full [720000, 256] f32."""
    full = np.empty((N_SAMPLES * M_TOTAL, N_COMB), dtype=np.float32)
    for i, o in enumerate(core_outs):
        # o[h, c, r] = out[r, 128h+c] -> [90000, 256]
        o = np.asarray(o).reshape(N_COMB, ROWS).T.astype(np.float32)
        for l in range(N_ANG):
            n = S_CORE * (2 * l + 1)
            src0 = S_CORE * l * l                      # local block offset
            dst0 = N_SAMPLES * l * l + i * n           # global block offset
            full[dst0:dst0 + n] = o[src0:src0 + n]
    return full


def run_sharded(in_maps, **kwargs):
    nc = build_nc()
    return run_bass_kernel_spmd(nc, in_maps, core_ids=list(range(N_CORES)),
                                **kwargs)


def kernel(**inputs):
    res = run_sharded(shard_inputs(inputs))
    return unshard_output([r["out"] for r in res.results])


# revision 10
# speedup vs baseline: 1.1761x; 1.0567x over previous
"""Trainium2 Bass kernel for nn_CombineRadialSpeciesWithAngular.

Per-angular-order GEMM out_l = v_l @ W[l], flattened+concatenated over l.
Full shapes: v_l [20000, 2l+1, 128] f32 (l=0..5), W [6, 128, 256] f32,
out [720000, 256] f32.

Strategy (8 NeuronCores, data-parallel over samples):
  - Each core gets 2500 samples of every block -> 90000 output rows.
  - Host pre-transposes each core's rows into vt [128, 90000] bf16
    (contraction dim p on partitions, rows in natural block order so
    supertile s of 2500 rows is l-homogeneous with l = isqrt(s)).
  - Device: stationary = W[l] half [128,128] bf16 (FWL weight loads),
    moving = 500-row slices of vt -> PSUM [128,500] f32 -> DVE/ACT copy
    to bf16 -> out_t [2,128,90000] bf16 (combined dim on partitions).
  - Host un-transposes out_t -> [90000, 256] f32 and re-concatenates.

Why this layout: every DMA (in and out) is 128 descriptors of 10 KB.
The HWDGE splits a transfer across SDMA engines only in equal parts, so
a 125-descriptor transfer lands on 5 engines (26 GB/s each) while a
128-descriptor one uses all 16. The f32 predecessor of this kernel was
DMA-bound at 170 GB/s effective for exactly that reason. bf16 I/O
additionally halves the bytes: 69 MB/core vs 138 MB/core, ~3e-3 rel
err from input/output rounding (gate is 2e-2). Input DMAs ride the
ACT HWDGE ring, output DMAs the SP ring, so the streams overlap.

Uses bacc.Bacc (not bass.Bass): its compile pipeline legalizes semaphore
waits to this target's 1-wait-per-instruction limit; plain Bass output
fails walrus codegen ("Too many sync wait commands").
"""

import math
import sys

import numpy as np

for _p in ("/opt/trn_rl_repo", "/root/.axon_site/_ro/trn_rl_repo"):
    if _p not in sys.path:
        sys.path.append(_p)

import ml_dtypes

import concourse.bacc as bacc
import concourse.mybir as mybir
import concourse.tile as tile
from concourse.bass_utils import run_bass_kernel_spmd

N_CORES = 8
N_SAMPLES = 20000
N_PROPS = 128
N_COMB = 256
N_ANG = 6
S_CORE = N_SAMPLES // N_CORES          # 2500 samples per core
M_TOTAL = sum(2 * l + 1 for l in range(N_ANG))  # 36
ROWS = S_CORE * M_TOTAL                # 90000 rows per core
SUP = 2500                             # rows per supertile (l-homogeneous)
NSUP = ROWS // SUP                     # 36
# Row-blocks per supertile: PSUM-bank-native sizes (512 f32 = one 2 KiB
# bank) so matmul outputs and copies are contiguous and bank-aligned.
BLOCKS = [(0, 512), (512, 512), (1024, 512), (1536, 512), (2048, 452)]
GROUPS = ((0, 1), (2, 3), (4,))        # blocks per PSUM tile / copy
PAIR = 2 * SUP                         # rows per input DMA (1.28 MB bf16)

F32 = mybir.dt.float32
BF16 = mybir.dt.bfloat16
NP_BF16 = ml_dtypes.bfloat16

_nc_cache = {}


def build_nc(reps=1):
    """reps>1 repeats the whole body inside one NEFF (profiling only)."""
    if reps in _nc_cache:
        return _nc_cache[reps]

    nc = bacc.Bacc()
    vt = nc.dram_tensor("vt", [128, ROWS], BF16, kind="ExternalInput")
    # w2[p, 2l+h, c] = W[l][p, 128h+c]
    w2 = nc.dram_tensor("w2", [128, 2 * N_ANG, 128], BF16, kind="ExternalInput")
    # out_t[h, c, r] = out[r, 128h+c]
    out = nc.dram_tensor("out", [2, 128, ROWS], BF16, kind="ExternalOutput")

    with tile.TileContext(nc) as tc:
        with (
            tc.tile_pool(name="wp", bufs=1) as wp,
            tc.tile_pool(name="vp", bufs=4) as vp,
            tc.tile_pool(name="op", bufs=2) as op,
            tc.tile_pool(name="pp", bufs=3, space="PSUM") as pp,
        ):
            wt = wp.tile([128, 2 * N_ANG, 128], BF16)
            nc.sync.dma_start(wt[:], w2[:])

            for rep in range(reps):
                for sp in range(NSUP // 2):           # supertile pairs
                    vt_t = vp.tile([128, PAIR], BF16)
                    # SWDGE ring: keeps input issue off the compute
                    # engines so a buffer-wait never stalls copies
                    nc.gpsimd.dma_start(
                        vt_t[:], vt[:, sp * PAIR:(sp + 1) * PAIR])

                    ot = [op.tile([128, PAIR], BF16, name=f"ot{h}")
                          for h in range(2)]
                    for half in range(2):
                        s = 2 * sp + half
                        l = math.isqrt(s)             # block bounds at squares
                        base = half * SUP
                        for h in range(2):
                            for gi, grp in enumerate(GROUPS):
                                ps = pp.tile([128, len(grp), 512], F32)
                                for q, b in enumerate(grp):
                                    off, n = BLOCKS[b]
                                    nc.tensor.matmul(
                                        ps[:, q, :n],
                                        wt[:, 2 * l + h, :],
                                        vt_t[:, base + off:base + off + n],
                                        start=True, stop=True)
                                off0 = BLOCKS[grp[0]][0]
                                tot = sum(BLOCKS[b][1] for b in grp)
                                dst = ot[h][:, base + off0:base + off0 + tot]
                                src = (ps[:, :, :] if tot == 1024
                                       else ps[:, 0, :tot])
                                # balance PSUM drain: DVE ~123 G elem/s,
                                # ACT ~114 G elem/s (cayman errata rates).
                                # All out DMAs stay on the idle SP engine:
                                # a dma_start on ACT would stall ACT's
                                # strict-FIFO queue at the wait-for-copy
                                # semaphore, blocking its own copies.
                                use_dve = (gi == 0) if gi < 2 else (
                                    (s + h) % 2 == 0)
                                if use_dve:
                                    nc.vector.tensor_copy(dst, src)
                                else:
                                    nc.scalar.copy(dst, src)

                    for h in range(2):
                        nc.sync.dma_start(
                            out[h, :, sp * PAIR:(sp + 1) * PAIR], ot[h][:])

    nc.finalize()  # Bacc compile: wait legalization + reg alloc
    _nc_cache[reps] = nc
    return nc


def shard_inputs(inputs):
    """Full f32 inputs -> per-core bf16 in_maps (host transpose + cast)."""
    w = np.asarray(inputs["W"], dtype=np.float32)          # [6,128,256]
    w2 = np.ascontiguousarray(
        w.transpose(1, 0, 2).reshape(128, N_ANG, 2, 128)
        .reshape(128, 2 * N_ANG, 128)).astype(NP_BF16)
    in_maps = []
    for i in range(N_CORES):
        vt_i = np.empty((128, ROWS), dtype=np.float32)
        col = 0
        for l in range(N_ANG):
            n = S_CORE * (2 * l + 1)
            blk = np.asarray(inputs[f"values_l{l}"][i * S_CORE:(i + 1) * S_CORE],
                             dtype=np.float32)
            vt_i[:, col:col + n] = blk.reshape(n, 128).T
            col += n
        in_maps.append({"vt": vt_i.astype(NP_BF16), "w2": w2})
    return in_maps


def unshard_output(core_outs):
    """Per-core [2, 128, 90000] bf16 -> full [720000, 256] f32."""
    full = np.empty((N_SAMPLES * M_TOTAL, N_COMB), dtype=np.float32)
    for i, o in enumerate(core_outs):
        # o[h, c, r] = out[r, 128h+c] -> [90000, 256]
        o = np.asarray(o).reshape(N_COMB, ROWS).T.astype(np.float32)
        for l in range(N_ANG):
            n = S_CORE * (2 * l + 1)
            src0 = S_CORE * l * l                      # local block offset
            dst0 = N_SAMPLES * l * l + i * n           # global block offset
            full[dst0:dst0 + n] = o[src0:src0 + n]
    return full


def run_sharded(in_maps, **kwargs):
    nc = build_nc()
    return run_bass_kernel_spmd(nc, in_maps, core_ids=list(range(N_CORES)),
                                **kwargs)


def kernel(**inputs):
    res = run_sharded(shard_inputs(inputs))
    return unshard_output([r["out"] for r in res.results])
